# revision 35
# baseline (speedup 1.0000x reference)
"""Distributed Trainium2 attention kernel (8 NeuronCores).

Strategy: tensor-parallel over heads (4 q heads + 1 shared KV head per
core), AllToAll to row-sharding, per-core output projection with full wo.

v2 layout: the causal-attention chains (phase C) are *injected* between
the QKV-projection matmuls (phase B) so the softmax-exp latency hides
under dense PE work; per-step rowsums moved off the TensorEngine onto
DVE SBUF accumulation; causal trim applied as a post-exp 0/1 mask-mul in
SBUF; PSUM evacuation via the Scalar engine (DVE PSUM reads measured
~15x slow); output projection split into two accumulation passes with a
bf16 partial save so the second AllToAll hides completely under pass 1;
AllToAll-2-dependent loads moved to the gpsimd DMA queue so they never
block weight-tile loads.
"""

import numpy as np
import ml_dtypes

import concourse.bass as bass
import concourse.mybir as mybir
import concourse.tile as tile
from concourse import bacc
from concourse import bass_utils

B, S, D = 2, 2048, 4096
H, HKV, HD = 32, 8, 128
HD2 = HD // 2
NC = 8
HL = H // NC            # 4 local q heads per core
BS = B * S              # 4096 global rows
R = BS // NC            # 512 output rows per core
NRB = BS // 128         # 32 row blocks
NDT = D // 128          # 32 contraction tiles
SCALE = 1.0 / float(np.sqrt(HD))
BF = mybir.dt.bfloat16
F32 = mybir.dt.float32

PROFILE = False         # set by test.py for neuron-profile capture
TMPDIR = None           # set by test.py to keep the trace dir


def _emit(nc, tc, io):
    xT, wqkvT, woT, ccR, mask01, onec, iden, out = io

    with (
        tc.tile_pool(name="wbuf", bufs=1) as wbuf,
        tc.tile_pool(name="qbuf", bufs=1) as qbuf,
        tc.tile_pool(name="kvbuf", bufs=1) as kvbuf,
        tc.tile_pool(name="abuf", bufs=1) as abuf,
        tc.tile_pool(name="cbuf", bufs=1) as cbuf,
        tc.tile_pool(name="xs", bufs=36) as xs,
        tc.tile_pool(name="cs", bufs=3) as cs,
        tc.tile_pool(name="es", bufs=8) as es,
        tc.tile_pool(name="rsb", bufs=3) as rsb,
        tc.tile_pool(name="ws", bufs=8) as ws,
        tc.tile_pool(name="ts", bufs=4) as ts,
        tc.tile_pool(name="ans", bufs=2) as ans,
        tc.tile_pool(name="rsp", bufs=2) as rsp,
        tc.tile_pool(name="os", bufs=2) as osp,
        tc.tile_pool(name="dram", bufs=1, space="DRAM") as dram,
    ):
        # ---- constants ----
        mask_sb = cbuf.tile([128, 128], BF, tag="tm")
        nc.sync.dma_start(mask_sb[:], mask01[:])
        onec_sb = cbuf.tile([128, 1], BF, tag="oc")
        nc.sync.dma_start(onec_sb[:], onec[:])
        iden_sb = cbuf.tile([128, 128], BF, tag="idn")
        nc.sync.dma_start(iden_sb[:], iden[:])

        # pre-load the exp table set so the first real exp doesn't pay ~2.7us
        warm = rsp.tile([1, 8], F32, tag="rc")
        nc.scalar.activation(
            warm[:, 0:1], onec_sb[0:1, 0:1],
            mybir.ActivationFunctionType.Exp,
        )
        # warm the collective stream: the first cc op pays ~11.5us of
        # trigger->start delay; a tiny dummy AllToAll absorbs it during B
        cw_in = dram.tile([8, 8], BF, name="ccw_in")
        cw_out = dram.tile([8, 8], BF, name="ccw_out")
        nc.sync.dma_start(cw_in[:], iden[0:8, 0:8])
        nc.gpsimd.collective_compute(
            "AllToAll",
            mybir.AluOpType.bypass,
            replica_groups=[list(range(NC))],
            ins=[cw_in.opt()],
            outs=[cw_out.opt()],
        )

        # resident QKV weights: col = dt*768 + [0:512 q | 512:640 k | 640:768 v]
        # spread chunks over 4 DMA queues so rb0's tail chunks arrive in time
        w_sb = wbuf.tile([128, NDT * 768], BF, tag="w")
        for dt in range(NDT):
            weng = (nc.sync, nc.scalar)[dt % 2]
            weng.dma_start(
                w_sb[:, dt * 768: dt * 768 + 768],
                wqkvT[dt * 128: (dt + 1) * 128, :],
            )

        q_sb = qbuf.tile([128, HL * BS], BF, tag="q")     # col = h*4096 + row
        kT_sb = kvbuf.tile([128, BS], BF, tag="k")        # col = row
        v_sb = kvbuf.tile([128, BS], BF, tag="v")         # col = rb*128 + hd

        # AllToAll halves by head pairs: in1 = heads {0,1}, in2 = {2,3}
        a2a_in1 = dram.tile([BS // 2, R], BF, name="a2a_in1")
        a2a_out1 = dram.tile([BS // 2, R], BF, name="a2a_out1")
        a2a_in2 = dram.tile([BS // 2, R], BF, name="a2a_in2")
        a2a_out2 = dram.tile([BS // 2, R], BF, name="a2a_out2")

        # ---- PSUM pools (B/C phase): 4 + 2 + 2 = 8 banks exactly ----
        psb = tc.alloc_tile_pool(name="psb", bufs=4, space="PSUM")
        psc = tc.alloc_tile_pool(name="psc", bufs=2, space="PSUM")

        # ---- phase C chain: one causal-attention chain as a generator ----
        # Yields once per emitted PE-step so the stripe loop can interleave
        # chain steps between QKV matmuls (the exp latency then hides under
        # B-phase PE work instead of stalling the in-order PE queue).
        def attn_chain(b, h, ci):
            qbase = h * BS + b * S
            ps_attn = psc.tile([128, 512], F32, tag="at", name=f"pa{b}{h}{ci}")
            rs_acc = rsb.tile([128, 512], F32, tag="rs", name=f"rs{b}{h}{ci}")
            jmax = 4 * ci + 3

            def qspan(j):
                q0 = max(j * 128, 512 * ci)
                return q0, 512 * ci + 512 - q0

            def pv(j, et, eoff, w, last):
                q0 = max(j * 128, 512 * ci)
                poff = q0 - 512 * ci
                kcol = (b * 16 + j) * 128
                nc.tensor.matmul(
                    ps_attn[:, poff: poff + w],
                    v_sb[:, kcol: kcol + 128],
                    et[:, eoff: eoff + w],
                    start=(j == 0), stop=last,
                )

            # per-j scores (f32, double-buffered psum) + per-j exp; the
            # paired/single-buffered variant serialized the two live chains
            prev = None
            for j in range(jmax + 1):
                q0, w = qspan(j)
                off = q0 - 512 * ci
                ps_s = psc.tile([128, 512], F32, tag="s", name=f"s{b}{h}{ci}{j}")
                nc.tensor.matmul(
                    ps_s[:, 0:w],
                    kT_sb[:, (b * 16 + j) * 128: (b * 16 + j) * 128 + 128],
                    q_sb[:, qbase + q0: qbase + q0 + w],
                    start=True, stop=True,
                )
                et = es.tile([128, 512], BF, tag="e", name=f"e{b}{h}{ci}{j}")
                nc.scalar.activation(
                    et[:, 0:w], ps_s[:, 0:w],
                    mybir.ActivationFunctionType.Exp, scale=SCALE,
                )
                if j // 4 == ci:
                    # causal trim of the diagonal block: exp(-inf) = 0
                    nc.vector.tensor_mul(et[:, 0:128], et[:, 0:128], mask_sb[:])
                # rowsum partials accumulate on DVE (SBUF), not the PE
                if j == 0:
                    nc.vector.tensor_copy(rs_acc[:, 0:512], et[:, 0:512])
                else:
                    nc.vector.tensor_add(
                        rs_acc[:, off: off + w], rs_acc[:, off: off + w],
                        et[:, 0:w],
                    )
                yield
                if prev is not None:
                    pj, pet, pw = prev
                    pv(pj, pet, 0, pw, False)
                    yield
                prev = (j, et, w)

            pj, pet, pw = prev
            pv(pj, pet, 0, pw, True)
            yield
            # total rowsum = ones^T @ rs_acc (partition reduce), via bf16
            rs_bf = es.tile([128, 512], BF, tag="e", name=f"rb{b}{h}{ci}")
            nc.vector.tensor_copy(rs_bf[:], rs_acc[:])
            ps_rs = psb.tile([1, 512], F32, tag="b", name=f"pr{b}{h}{ci}")
            nc.tensor.matmul(ps_rs[:], onec_sb[:], rs_bf[:], start=True, stop=True)
            yield
            rc = rsp.tile([1, 512], F32, tag="rc", name=f"rc{b}{h}{ci}")
            nc.vector.reciprocal(rc[:], ps_rs[:])
            bc = rsp.tile([128, 512], F32, tag="bc", name=f"bc{b}{h}{ci}")
            nc.gpsimd.partition_broadcast(bc[:], rc[:])
            yield
            # evacuate PSUM on the Scalar engine (DVE PSUM reads are slow)
            attv = ans.tile([128, 512], BF, tag="av", name=f"av{b}{h}{ci}")
            nc.scalar.activation(
                attv[:], ps_attn[:], mybir.ActivationFunctionType.Copy
            )
            an = ans.tile([128, 512], BF, tag="an", name=f"an{b}{h}{ci}")
            nc.vector.tensor_mul(an[:], attv[:], bc[:])
            dst = a2a_in1 if h < 2 else a2a_in2
            blk = 256 * (b * 4 + ci) + 128 * (h % 2)
            nc.sync.dma_start(dst[blk: blk + 128, :], an[:])
            yield

        def group_actions(b, g):
            # 4 chains of group (b, ci=g): pairs (h0,h1) then (h2,h3),
            # each pair 2-way interleaved one action at a time.
            for ha, hb in ((0, 1), (2, 3)):
                gens = [attn_chain(b, ha, g), attn_chain(b, hb, g)]
                alive = [True, True]
                while any(alive):
                    for i, gen in enumerate(gens):
                        if not alive[i]:
                            continue
                        try:
                            next(gen)
                            yield
                        except StopIteration:
                            alive[i] = False

        # ---- phase B stripes with C injection ----
        def b_rope_tail_q(rb, ps_q, cct, sst):
            qe = ps_q[:].rearrange("p (h d) -> p h d", d=128)[:, :, 0:HD2]
            qo = ps_q[:].rearrange("p (h d) -> p h d", d=128)[:, :, HD2:HD]
            t1 = ts.tile([128, 256], BF, tag="t")
            t2 = ts.tile([128, 256], BF, tag="t")
            t3 = ts.tile([128, 256], BF, tag="t")
            t4 = ts.tile([128, 256], BF, tag="t")
            nc.vector.tensor_mul(t1[:], qe, cct[:])
            nc.vector.tensor_mul(t2[:], qo, sst[:])
            nc.vector.tensor_mul(t3[:], qe, sst[:])
            nc.vector.tensor_mul(t4[:], qo, cct[:])
            qrot = ts.tile([128, 512], BF, tag="qr", bufs=2)
            qre = qrot[:].rearrange("p (h d) -> p h d", d=128)[:, :, 0:HD2]
            qro = qrot[:].rearrange("p (h d) -> p h d", d=128)[:, :, HD2:HD]
            nc.vector.tensor_sub(qre, t1[:], t2[:])
            nc.vector.tensor_add(qro, t3[:], t4[:])
            return qrot

        def b_transpose_tail_q(rb, qrot):
            ps_tq = psc.tile([128, 512], BF, tag="s", name=f"tq{rb}")
            for h in range(HL):
                nc.tensor.transpose(
                    ps_tq[:, h * 128: (h + 1) * 128],
                    qrot[:, h * 128: (h + 1) * 128],
                    iden_sb[:],
                )
            q_dst = (
                q_sb[:]
                .rearrange("p (h r) -> p h r", h=HL)
                [:, :, rb * 128: (rb + 1) * 128]
            )
            nc.vector.tensor_copy(
                q_dst, ps_tq[:].rearrange("p (h r) -> p h r", h=HL)
            )

        def b_rope_tail_kv(rb, ps_kv, cct, sst):
            ke = ps_kv[:, 0:HD2]
            ko = ps_kv[:, HD2:HD]
            u1 = ts.tile([128, 64], BF, tag="u", bufs=4)
            u2 = ts.tile([128, 64], BF, tag="u", bufs=4)
            u3 = ts.tile([128, 64], BF, tag="u", bufs=4)
            u4 = ts.tile([128, 64], BF, tag="u", bufs=4)
            nc.vector.tensor_mul(u1[:], ke, cct[:, 0:HD2])
            nc.vector.tensor_mul(u2[:], ko, sst[:, 0:HD2])
            nc.vector.tensor_mul(u3[:], ke, sst[:, 0:HD2])
            nc.vector.tensor_mul(u4[:], ko, cct[:, 0:HD2])
            krot = ts.tile([128, 128], BF, tag="kr", bufs=2)
            nc.vector.tensor_sub(krot[:, 0:HD2], u1[:], u2[:])
            nc.vector.tensor_add(krot[:, HD2:HD], u3[:], u4[:])
            # v: plain copy to row-major storage (Scalar engine)
            nc.scalar.activation(
                v_sb[:, rb * 128: (rb + 1) * 128], ps_kv[:, 128:256],
                mybir.ActivationFunctionType.Copy,
            )
            return krot

        def b_transpose_tail_kv(rb, krot):
            ps_tk = psc.tile([128, 128], BF, tag="s", name=f"tk{rb}")
            nc.tensor.transpose(ps_tk[:], krot[:], iden_sb[:])
            nc.vector.tensor_copy(kT_sb[:, rb * 128: (rb + 1) * 128], ps_tk[:])

        from collections import deque
        pending_actions = deque()   # C actions awaiting injection

        def pump(k=1):
            for _ in range(k):
                if not pending_actions:
                    return
                gen = pending_actions[0]
                try:
                    next(gen)
                except StopIteration:
                    pending_actions.popleft()

        # inject C steps at most dts; keep the rope/transpose tail dts clean
        INJ = set(range(NDT)) - {0, 2, 12}

        # x tiles as [128, 256] PAIR-tiles (one DMA serves two row blocks):
        # DMA trigger instructions cost ~650ns of engine-queue time each,
        # so trigger count (not bytes) is the scarce resource. A pair's
        # tiles are fetched one rb ahead (during the previous pair's 2nd
        # rb), spread over all three DMA-capable queues.
        def x2_fetch(pr, dt):
            xt = xs.tile([128, 256], BF, tag="x", name=f"x{pr}_{dt}")
            eng = (nc.sync, nc.scalar, nc.gpsimd)[dt % 3]
            eng.dma_start(
                xt[:], xT[dt * 128: (dt + 1) * 128, pr * 256: (pr + 1) * 256]
            )
            return xt

        xcur = [x2_fetch(0, dt) for dt in range(NDT)]
        xnext = []

        pending = None
        rot = None
        for rb in range(NRB):
            # rope tables for this rb's tail (used one rb later); cc and ss
            # interleaved host-side into one table -> a single DMA per rb
            ccs = cs.tile([128, 512], BF, tag="cc", name=f"cc{rb}")
            nc.sync.dma_start(ccs[:], ccR[:, rb * 512: (rb + 1) * 512])
            cct, sst = ccs[:, 0:256], ccs[:, 256:512]

            if rb % 2 == 0 and rb > 0:
                xcur = xnext
                xnext = []

            ps_q = psb.tile([128, 512], F32, tag="b", name=f"pq{rb}")
            ps_kv = psb.tile([128, 256], F32, tag="b", name=f"pkv{rb}")
            half = (rb % 2) * 128
            for dt in range(NDT):
                xt = xcur[dt][:, half: half + 128]
                if rb % 2 == 1 and rb + 1 < NRB:
                    xnext.append(x2_fetch((rb + 1) // 2, dt))
                st, sp = dt == 0, dt == NDT - 1
                nc.tensor.matmul(
                    ps_q[:], xt[:], w_sb[:, dt * 768: dt * 768 + 512],
                    start=st, stop=sp,
                )
                nc.tensor.matmul(
                    ps_kv[:], xt[:], w_sb[:, dt * 768 + 512: dt * 768 + 768],
                    start=st, stop=sp,
                )
                if dt == 2 and pending is not None:
                    prb, pq, pkv, pcc, pss = pending
                    rot = (prb, b_rope_tail_q(prb, pq, pcc, pss),
                           b_rope_tail_kv(prb, pkv, pcc, pss))
                    pending = None
                if dt == 12 and rot is not None:
                    b_transpose_tail_q(rot[0], rot[1])
                    b_transpose_tail_kv(rot[0], rot[2])
                    rot = None
                    # stripe boundary: after the transposes of the last rb
                    # of stripe g land, group g's chains become runnable
                    if rb % 4 == 0 and rb >= 4:
                        g = rb // 4 - 1
                        pending_actions.append(group_actions(g // 4, g % 4))
                if dt in INJ:
                    # drain faster when backlogged (late, larger groups)
                    pump(2 if len(pending_actions) > 1 or rb >= 24 else 1)
            pending = (rb, ps_q, ps_kv, cct, sst)

        # flush the last rb's tails
        prb, pq, pkv, pcc, pss = pending
        rot = (prb, b_rope_tail_q(prb, pq, pcc, pss),
               b_rope_tail_kv(prb, pkv, pcc, pss))
        b_transpose_tail_q(rot[0], rot[1])
        b_transpose_tail_kv(rot[0], rot[2])

        # drain any carryover, then the tail group (b1, ci=3)
        while pending_actions:
            pump(16)

        def a2a_guard(buf, tag):
            # DRAM round-trip touching one row of every 128-row an block:
            # the write-back is a tracked writer of the collective's input,
            # so the collective cannot start until all an data is visible
            # in DRAM (guards a rare input-corruption flake).
            tok = cs.tile([16, 4], BF, tag="tok", bufs=2, name=f"tok{tag}")
            blk_ap = (
                buf[:].rearrange("(blk p) c -> blk p c", p=128)[:, 127:128, 0:4]
            )
            tok_ap = tok[:].rearrange("p (one c) -> p one c", one=1)
            nc.sync.dma_start(tok_ap, blk_ap)
            nc.sync.dma_start(blk_ap, tok_ap)

        for ha, hb in ((0, 1), (2, 3)):
            gens = [attn_chain(1, ha, 3), attn_chain(1, hb, 3)]
            alive = [True, True]
            while any(alive):
                for i, gen in enumerate(gens):
                    if alive[i]:
                        try:
                            next(gen)
                        except StopIteration:
                            alive[i] = False
            if ha == 0:
                a2a_guard(a2a_in1, "a")
                nc.gpsimd.collective_compute(
                    "AllToAll",
                    mybir.AluOpType.bypass,
                    replica_groups=[list(range(NC))],
                    ins=[a2a_in1.opt()],
                    outs=[a2a_out1.opt()],
                )
        a2a_guard(a2a_in2, "b")
        nc.gpsimd.collective_compute(
            "AllToAll",
            mybir.AluOpType.bypass,
            replica_groups=[list(range(NC))],
            ins=[a2a_in2.opt()],
            outs=[a2a_out2.opt()],
        )

        psc.release()
        psb.release()

        # ---- phase D: output projection, two accumulation passes ----
        # pass 1 covers the a2a_out1 head-tiles for all col groups (hiding
        # AllToAll#2 under ~109us of PE work), saves bf16 partials, then
        # pass 2 accumulates the a2a_out2 tiles and merges. Head tiles are
        # ordered in adjacent pairs so one DMA loads two wo row-blocks.
        ht1 = [4 * i + l for i in range(8) for l in (0, 1)]
        ht2 = [4 * i + l for i in range(8) for l in (2, 3)]
        at_sb = abuf.tile([128, 32 * 512], BF, tag="at")  # col = ht*512 + row
        part_sb = wbuf.tile([128, 32 * 512], BF, tag="w")  # reuse w_sb slot

        def load_wt2(cg, ht, queue):
            # one DMA for two adjacent 128-row wo blocks -> [128, 2x512]
            wt2 = ws.tile([128, 1024], BF, tag="wo")
            queue.dma_start(
                wt2[:].rearrange("p (two c) -> p two c", two=2),
                woT[ht * 128: ht * 128 + 256, cg * 512: (cg + 1) * 512]
                .rearrange("(two p) c -> p two c", two=2),
            )
            return wt2

        def at_load(ht, src_buf, queue):
            i, htl = ht // 4, ht % 4
            srow = (i * 2 + (htl % 2)) * 128
            queue.dma_start(
                at_sb[:, ht * 512: (ht + 1) * 512],
                src_buf[srow: srow + 128, :],
            )

        with tc.tile_pool(name="psd", bufs=8, space="PSUM") as psd:
            # hoisted wo prefetch for pass-1 cg0: these DMAs depend only on
            # woT, so they run during the AllToAll-1 wait
            wt_pre = {}
            for m in range(8):
                wt_pre[(0, m)] = load_wt2(0, ht1[2 * m], (nc.sync, nc.scalar)[m % 2])
            for idx, ht in enumerate(ht1[:4]):
                at_load(ht, a2a_out1, (nc.sync, nc.scalar)[idx % 2])
            for ht in ht2:
                # gpsimd queue: waits on AllToAll#2 w/o blocking weight loads
                at_load(ht, a2a_out2, nc.gpsimd)

            def dpass(hts, src1, pidx):
                for cg in range(8):
                    po = [
                        psd.tile([128, 512], F32, tag="d", name=f"po{pidx}_{cg}_{i}")
                        for i in range(4)
                    ]
                    wt2 = None
                    for n, ht in enumerate(hts):
                        m = n // 2
                        if n % 2 == 0:
                            wt2 = wt_pre.pop((cg, m), None)
                            if wt2 is None:
                                wt2 = load_wt2(
                                    cg, ht, (nc.sync, nc.scalar)[(cg + m) % 2]
                                )
                            # prefetch the next pair's weights ahead
                            # (a just-in-time load misses by ~1us)
                            nm = m + 1
                            ncg = cg + nm // 8
                            nm = nm % 8
                            if ncg < 8 and (ncg, nm) not in wt_pre:
                                wt_pre[(ncg, nm)] = load_wt2(
                                    ncg, hts[2 * nm],
                                    (nc.sync, nc.scalar)[(ncg + nm) % 2],
                                )
                        if pidx == 1 and cg == 0 and n + 4 < 16:
                            at_load(hts[n + 4], src1, (nc.sync, nc.scalar)[n % 2])
                        half = (n % 2) * 512
                        for rt in range(4):
                            nc.tensor.matmul(
                                po[rt][:],
                                at_sb[:, ht * 512 + rt * 128: ht * 512 + (rt + 1) * 128],
                                wt2[:, half: half + 512],
                                start=(n == 0), stop=(n == 15),
                            )
                    for rt in range(4):
                        if pidx == 1:
                            nc.scalar.activation(
                                part_sb[:, (cg * 4 + rt) * 512: (cg * 4 + rt + 1) * 512],
                                po[rt][:],
                                mybir.ActivationFunctionType.Copy,
                            )
                        else:
                            tmp = osp.tile([128, 512], BF, tag="tmp")
                            nc.scalar.activation(
                                tmp[:], po[rt][:],
                                mybir.ActivationFunctionType.Copy,
                            )
                            ot = osp.tile([128, 512], F32, tag="o")
                            nc.vector.tensor_add(
                                ot[:], tmp[:],
                                part_sb[:, (cg * 4 + rt) * 512: (cg * 4 + rt + 1) * 512],
                            )
                            nc.sync.dma_start(
                                out[rt * 128: (rt + 1) * 128,
                                    cg * 512: (cg + 1) * 512],
                                ot[:],
                            )

            dpass(ht1, a2a_out1, 1)
            for m2 in range(4):
                wt_pre[(0, m2)] = load_wt2(
                    0, ht2[2 * m2], (nc.sync, nc.scalar)[m2 % 2]
                )
            dpass(ht2, a2a_out2, 2)


_LDW_PATCHED = False


def _patch_ldw_opt():
    """Enable walrus's redundant-LDWEIGHTS elision (off by default in
    concourse's compile flags; our phase-B matmul pairs share the same
    stationary operand back to back)."""
    global _LDW_PATCHED
    if _LDW_PATCHED:
        return
    _LDW_PATCHED = True
    real_run = bass_utils.run_command

    def run_hook(argv, **kw):
        argv = [
            a.replace("--enable-ldw-opt=false", "--enable-ldw-opt=true")
            if isinstance(a, str) else a
            for a in argv
        ]
        return real_run(argv, **kw)

    bass_utils.run_command = run_hook


def _build():
    # note: walrus's LDW-elision (--enable-ldw-opt=true) rejects the
    # interleaved chain/projection instruction stream; leave it off.
    nc = bacc.Bacc("TRN2", target_bir_lowering=False, debug=False, num_devices=NC)
    xT = nc.dram_tensor("xT", [D, BS], BF, kind="ExternalInput")
    wqkvT = nc.dram_tensor("wqkvT", [D, 768], BF, kind="ExternalInput")
    woT = nc.dram_tensor("woT", [D, D], BF, kind="ExternalInput")
    ccR = nc.dram_tensor("ccR", [128, NRB * 512], BF, kind="ExternalInput")
    mask01 = nc.dram_tensor("mask01", [128, 128], BF, kind="ExternalInput")
    onec = nc.dram_tensor("onec", [128, 1], BF, kind="ExternalInput")
    iden = nc.dram_tensor("iden", [128, 128], BF, kind="ExternalInput")
    out = nc.dram_tensor("out", [R, D], F32, kind="ExternalOutput")
    with tile.TileContext(nc) as tc:
        _emit(nc, tc, (xT, wqkvT, woT, ccR, mask01, onec, iden, out))
    nc.compile()
    return nc


_NC = None


def kernel(x, wq, wk, wv, wo, freqs_cos, freqs_sin, mask, start_pos):
    global _NC
    if _NC is None:
        _NC = _build()
    nc = _NC
    bf = ml_dtypes.bfloat16

    x = np.asarray(x, dtype=np.float32)
    xT = np.ascontiguousarray(x.reshape(BS, D).T).astype(bf)

    perm = np.concatenate([np.arange(0, HD, 2), np.arange(1, HD, 2)])
    wqTp = np.asarray(wq, np.float32).T.reshape(D, H, HD)[:, :, perm]
    wkTp = np.asarray(wk, np.float32).T.reshape(D, HKV, HD)[:, :, perm]
    wvT = np.asarray(wv, np.float32).T.reshape(D, HKV, HD)
    woT = np.ascontiguousarray(np.asarray(wo, np.float32).T).astype(bf)

    fc = np.asarray(freqs_cos, np.float32)
    fs = np.asarray(freqs_sin, np.float32)
    # row-major RoPE tables per row block, replicated x4 along free axis
    pos = (np.arange(BS) % S).reshape(NRB, 128)
    ccR4 = np.tile(fc[pos], (1, 1, 4)).transpose(1, 0, 2).reshape(128, NRB, 256)
    ssR4 = np.tile(fs[pos], (1, 1, 4)).transpose(1, 0, 2).reshape(128, NRB, 256)
    # interleave per-rb: [cc 256 | ss 256] so one DMA loads both tables
    ccR = np.ascontiguousarray(
        np.concatenate([ccR4, ssR4], axis=2).reshape(128, NRB * 512)
    ).astype(bf)

    # 0/1 causal mask for the diagonal block (krow <= qcol keeps)
    mask01 = np.where(
        np.arange(128)[:, None] > np.arange(128)[None, :], 0.0, 1.0
    ).astype(bf)
    onec = np.ones((128, 1), dtype=bf)
    iden = np.eye(128, dtype=bf)

    in_maps = []
    for c in range(NC):
        wqkv = np.concatenate(
            [
                wqTp[:, 4 * c: 4 * c + 4].reshape(D, 512),
                wkTp[:, c],
                wvT[:, c],
            ],
            axis=1,
        ).astype(bf)
        in_maps.append(
            {
                "xT": xT,
                "wqkvT": np.ascontiguousarray(wqkv),
                "woT": woT,
                "ccR": ccR,
                "mask01": mask01,
                "onec": onec,
                "iden": iden,
            }
        )

    res = bass_utils.run_bass_kernel_spmd(
        nc, in_maps, core_ids=list(range(NC)), trace=PROFILE, tmpdir=TMPDIR
    )
    if PROFILE:
        print(f"HW exec time: {res.exec_time_ns} ns")
        if res.instructions_and_trace is not None:
            print(f"trace: {res.instructions_and_trace[1]}")

    out_full = np.empty((BS, D), dtype=np.float32)
    for c in range(NC):
        out_full[R * c: R * (c + 1)] = res.results[c]["out"]
    return out_full.reshape(B, S, D)


# revision 36
# speedup vs baseline: 1.0784x; 1.0784x over previous
"""Distributed Trainium2 attention kernel (8 NeuronCores).

Strategy: tensor-parallel over heads (4 q heads + 1 shared KV head per
core), AllToAll to row-sharding, per-core output projection with full wo.

Schedule: the causal-attention chains (phase C) are *injected* between
the QKV-projection matmuls (phase B) so the softmax-exp latency hides
under dense PE work; per-step rowsums run on DVE SBUF accumulation
instead of the TensorEngine; causal trim is a post-exp 0/1 mask-mul in
SBUF; PSUM is evacuated via the Scalar engine (DVE PSUM reads measured
~15x slow); the output projection is split into two accumulation passes
with a bf16 partial save so the second AllToAll hides under pass-1 PE
work; AllToAll-2-dependent loads sit on the gpsimd DMA queue so they
never block weight-tile loads; DMA trigger instructions cost ~650ns of
queue time each, so x/rope/wo tiles are fetched with merged DMAs spread
across the three DMA-capable queues; a dummy collective absorbs the
~11.5us first-collective delay; a DRAM round-trip guards the AllToAll
inputs against a rare write-visibility flake.
"""

import numpy as np
import ml_dtypes

import concourse.bass as bass
import concourse.mybir as mybir
import concourse.tile as tile
from concourse import bacc
from concourse import bass_utils

B, S, D = 2, 2048, 4096
H, HKV, HD = 32, 8, 128
HD2 = HD // 2
NC = 8
HL = H // NC            # 4 local q heads per core
BS = B * S              # 4096 global rows
R = BS // NC            # 512 output rows per core
NRB = BS // 128         # 32 row blocks
NDT = D // 128          # 32 contraction tiles
SCALE = 1.0 / float(np.sqrt(HD))
BF = mybir.dt.bfloat16
F32 = mybir.dt.float32

PROFILE = False         # set by test.py for neuron-profile capture
TMPDIR = None           # set by test.py to keep the trace dir


def _emit(nc, tc, io):
    xT, wqkvT, woT, ccR, mask01, onec, iden, out = io

    with (
        tc.tile_pool(name="wbuf", bufs=1) as wbuf,
        tc.tile_pool(name="qbuf", bufs=1) as qbuf,
        tc.tile_pool(name="kvbuf", bufs=1) as kvbuf,
        tc.tile_pool(name="abuf", bufs=1) as abuf,
        tc.tile_pool(name="cbuf", bufs=1) as cbuf,
        tc.tile_pool(name="xs", bufs=36) as xs,
        tc.tile_pool(name="cs", bufs=3) as cs,
        tc.tile_pool(name="es", bufs=8) as es,
        tc.tile_pool(name="rsb", bufs=3) as rsb,
        tc.tile_pool(name="ws", bufs=8) as ws,
        tc.tile_pool(name="ts", bufs=4) as ts,
        tc.tile_pool(name="ans", bufs=2) as ans,
        tc.tile_pool(name="rsp", bufs=2) as rsp,
        tc.tile_pool(name="os", bufs=2) as osp,
        tc.tile_pool(name="dram", bufs=1, space="DRAM") as dram,
    ):
        # ---- constants ----
        mask_sb = cbuf.tile([128, 128], BF, tag="tm")
        nc.sync.dma_start(mask_sb[:], mask01[:])
        onec_sb = cbuf.tile([128, 1], BF, tag="oc")
        nc.sync.dma_start(onec_sb[:], onec[:])
        iden_sb = cbuf.tile([128, 128], BF, tag="idn")
        nc.sync.dma_start(iden_sb[:], iden[:])

        # pre-load the exp table set so the first real exp doesn't pay ~2.7us
        warm = rsp.tile([1, 8], F32, tag="rc")
        nc.scalar.activation(
            warm[:, 0:1], onec_sb[0:1, 0:1],
            mybir.ActivationFunctionType.Exp,
        )
        # warm the collective stream: the first cc op pays ~11.5us of
        # trigger->start delay; a tiny dummy AllToAll absorbs it during B
        cw_in = dram.tile([8, 8], BF, name="ccw_in")
        cw_out = dram.tile([8, 8], BF, name="ccw_out")
        nc.sync.dma_start(cw_in[:], iden[0:8, 0:8])
        nc.gpsimd.collective_compute(
            "AllToAll",
            mybir.AluOpType.bypass,
            replica_groups=[list(range(NC))],
            ins=[cw_in.opt()],
            outs=[cw_out.opt()],
        )

        # resident QKV weights: col = dt*768 + [0:512 q | 512:640 k | 640:768 v]
        # spread chunks over 4 DMA queues so rb0's tail chunks arrive in time
        w_sb = wbuf.tile([128, NDT * 768], BF, tag="w")
        for dt in range(NDT):
            weng = (nc.sync, nc.scalar)[dt % 2]
            weng.dma_start(
                w_sb[:, dt * 768: dt * 768 + 768],
                wqkvT[dt * 128: (dt + 1) * 128, :],
            )

        q_sb = qbuf.tile([128, HL * BS], BF, tag="q")     # col = h*4096 + row
        kT_sb = kvbuf.tile([128, BS], BF, tag="k")        # col = row
        v_sb = kvbuf.tile([128, BS], BF, tag="v")         # col = rb*128 + hd

        # AllToAll halves by head pairs: in1 = heads {0,1}, in2 = {2,3}
        a2a_in1 = dram.tile([BS // 2, R], BF, name="a2a_in1")
        a2a_out1 = dram.tile([BS // 2, R], BF, name="a2a_out1")
        a2a_in2 = dram.tile([BS // 2, R], BF, name="a2a_in2")
        a2a_out2 = dram.tile([BS // 2, R], BF, name="a2a_out2")

        # ---- PSUM pools (B/C phase): 4 + 2 + 2 = 8 banks exactly ----
        psb = tc.alloc_tile_pool(name="psb", bufs=4, space="PSUM")
        psc = tc.alloc_tile_pool(name="psc", bufs=2, space="PSUM")

        # ---- phase C chain: one causal-attention chain as a generator ----
        # Yields once per emitted PE-step so the stripe loop can interleave
        # chain steps between QKV matmuls (the exp latency then hides under
        # B-phase PE work instead of stalling the in-order PE queue).
        def attn_chain(b, h, ci):
            qbase = h * BS + b * S
            ps_attn = psc.tile([128, 512], F32, tag="at", name=f"pa{b}{h}{ci}")
            rs_acc = rsb.tile([128, 512], F32, tag="rs", name=f"rs{b}{h}{ci}")
            jmax = 4 * ci + 3

            def qspan(j):
                q0 = max(j * 128, 512 * ci)
                return q0, 512 * ci + 512 - q0

            def pv(j, et, eoff, w, last):
                q0 = max(j * 128, 512 * ci)
                poff = q0 - 512 * ci
                kcol = (b * 16 + j) * 128
                nc.tensor.matmul(
                    ps_attn[:, poff: poff + w],
                    v_sb[:, kcol: kcol + 128],
                    et[:, eoff: eoff + w],
                    start=(j == 0), stop=last,
                )

            # per-j scores (f32, double-buffered psum) + per-j exp; the
            # paired/single-buffered variant serialized the two live chains
            prev = None
            for j in range(jmax + 1):
                q0, w = qspan(j)
                off = q0 - 512 * ci
                ps_s = psc.tile([128, 512], F32, tag="s", name=f"s{b}{h}{ci}{j}")
                nc.tensor.matmul(
                    ps_s[:, 0:w],
                    kT_sb[:, (b * 16 + j) * 128: (b * 16 + j) * 128 + 128],
                    q_sb[:, qbase + q0: qbase + q0 + w],
                    start=True, stop=True,
                )
                et = es.tile([128, 512], BF, tag="e", name=f"e{b}{h}{ci}{j}")
                nc.scalar.activation(
                    et[:, 0:w], ps_s[:, 0:w],
                    mybir.ActivationFunctionType.Exp, scale=SCALE,
                )
                if j // 4 == ci:
                    # causal trim of the diagonal block: exp(-inf) = 0
                    nc.vector.tensor_mul(et[:, 0:128], et[:, 0:128], mask_sb[:])
                # rowsum partials accumulate on DVE (SBUF), not the PE
                if j == 0:
                    nc.vector.tensor_copy(rs_acc[:, 0:512], et[:, 0:512])
                else:
                    nc.vector.tensor_add(
                        rs_acc[:, off: off + w], rs_acc[:, off: off + w],
                        et[:, 0:w],
                    )
                yield
                if prev is not None:
                    pj, pet, pw = prev
                    pv(pj, pet, 0, pw, False)
                    yield
                prev = (j, et, w)

            pj, pet, pw = prev
            pv(pj, pet, 0, pw, True)
            yield
            # total rowsum = ones^T @ rs_acc (partition reduce), via bf16
            rs_bf = es.tile([128, 512], BF, tag="e", name=f"rb{b}{h}{ci}")
            nc.vector.tensor_copy(rs_bf[:], rs_acc[:])
            ps_rs = psb.tile([1, 512], F32, tag="b", name=f"pr{b}{h}{ci}")
            nc.tensor.matmul(ps_rs[:], onec_sb[:], rs_bf[:], start=True, stop=True)
            yield
            rc = rsp.tile([1, 512], F32, tag="rc", name=f"rc{b}{h}{ci}")
            nc.vector.reciprocal(rc[:], ps_rs[:])
            bc = rsp.tile([128, 512], F32, tag="bc", name=f"bc{b}{h}{ci}")
            nc.gpsimd.partition_broadcast(bc[:], rc[:])
            yield
            # evacuate PSUM on the Scalar engine (DVE PSUM reads are slow)
            attv = ans.tile([128, 512], BF, tag="av", name=f"av{b}{h}{ci}")
            nc.scalar.activation(
                attv[:], ps_attn[:], mybir.ActivationFunctionType.Copy
            )
            an = ans.tile([128, 512], BF, tag="an", name=f"an{b}{h}{ci}")
            nc.vector.tensor_mul(an[:], attv[:], bc[:])
            dst = a2a_in1 if h < 2 else a2a_in2
            blk = 256 * (b * 4 + ci) + 128 * (h % 2)
            nc.sync.dma_start(dst[blk: blk + 128, :], an[:])
            yield

        def group_actions(b, g):
            # 4 chains of group (b, ci=g): pairs (h0,h1) then (h2,h3),
            # each pair 2-way interleaved one action at a time.
            for ha, hb in ((0, 1), (2, 3)):
                gens = [attn_chain(b, ha, g), attn_chain(b, hb, g)]
                alive = [True, True]
                while any(alive):
                    for i, gen in enumerate(gens):
                        if not alive[i]:
                            continue
                        try:
                            next(gen)
                            yield
                        except StopIteration:
                            alive[i] = False

        # ---- phase B stripes with C injection ----
        def b_rope_tail_q(rb, ps_q, cct, sst):
            qe = ps_q[:].rearrange("p (h d) -> p h d", d=128)[:, :, 0:HD2]
            qo = ps_q[:].rearrange("p (h d) -> p h d", d=128)[:, :, HD2:HD]
            t1 = ts.tile([128, 256], BF, tag="t")
            t2 = ts.tile([128, 256], BF, tag="t")
            t3 = ts.tile([128, 256], BF, tag="t")
            t4 = ts.tile([128, 256], BF, tag="t")
            nc.vector.tensor_mul(t1[:], qe, cct[:])
            nc.vector.tensor_mul(t2[:], qo, sst[:])
            nc.vector.tensor_mul(t3[:], qe, sst[:])
            nc.vector.tensor_mul(t4[:], qo, cct[:])
            qrot = ts.tile([128, 512], BF, tag="qr", bufs=2)
            qre = qrot[:].rearrange("p (h d) -> p h d", d=128)[:, :, 0:HD2]
            qro = qrot[:].rearrange("p (h d) -> p h d", d=128)[:, :, HD2:HD]
            nc.vector.tensor_sub(qre, t1[:], t2[:])
            nc.vector.tensor_add(qro, t3[:], t4[:])
            return qrot

        def b_transpose_tail_q(rb, qrot):
            ps_tq = psc.tile([128, 512], BF, tag="s", name=f"tq{rb}")
            for h in range(HL):
                nc.tensor.transpose(
                    ps_tq[:, h * 128: (h + 1) * 128],
                    qrot[:, h * 128: (h + 1) * 128],
                    iden_sb[:],
                )
            q_dst = (
                q_sb[:]
                .rearrange("p (h r) -> p h r", h=HL)
                [:, :, rb * 128: (rb + 1) * 128]
            )
            nc.vector.tensor_copy(
                q_dst, ps_tq[:].rearrange("p (h r) -> p h r", h=HL)
            )

        def b_rope_tail_kv(rb, ps_kv, cct, sst):
            ke = ps_kv[:, 0:HD2]
            ko = ps_kv[:, HD2:HD]
            u1 = ts.tile([128, 64], BF, tag="u", bufs=4)
            u2 = ts.tile([128, 64], BF, tag="u", bufs=4)
            u3 = ts.tile([128, 64], BF, tag="u", bufs=4)
            u4 = ts.tile([128, 64], BF, tag="u", bufs=4)
            nc.vector.tensor_mul(u1[:], ke, cct[:, 0:HD2])
            nc.vector.tensor_mul(u2[:], ko, sst[:, 0:HD2])
            nc.vector.tensor_mul(u3[:], ke, sst[:, 0:HD2])
            nc.vector.tensor_mul(u4[:], ko, cct[:, 0:HD2])
            krot = ts.tile([128, 128], BF, tag="kr", bufs=2)
            nc.vector.tensor_sub(krot[:, 0:HD2], u1[:], u2[:])
            nc.vector.tensor_add(krot[:, HD2:HD], u3[:], u4[:])
            # v: plain copy to row-major storage (Scalar engine)
            nc.scalar.activation(
                v_sb[:, rb * 128: (rb + 1) * 128], ps_kv[:, 128:256],
                mybir.ActivationFunctionType.Copy,
            )
            return krot

        def b_transpose_tail_kv(rb, krot):
            ps_tk = psc.tile([128, 128], BF, tag="s", name=f"tk{rb}")
            nc.tensor.transpose(ps_tk[:], krot[:], iden_sb[:])
            nc.vector.tensor_copy(kT_sb[:, rb * 128: (rb + 1) * 128], ps_tk[:])

        from collections import deque
        pending_actions = deque()   # C actions awaiting injection

        def pump(k=1):
            for _ in range(k):
                if not pending_actions:
                    return
                gen = pending_actions[0]
                try:
                    next(gen)
                except StopIteration:
                    pending_actions.popleft()

        # inject C steps at most dts; keep the rope/transpose tail dts clean
        INJ = set(range(NDT)) - {0, 2, 12}

        # x tiles as [128, 256] PAIR-tiles (one DMA serves two row blocks):
        # DMA trigger instructions cost ~650ns of engine-queue time each,
        # so trigger count (not bytes) is the scarce resource. A pair's
        # tiles are fetched one rb ahead (during the previous pair's 2nd
        # rb), spread over all three DMA-capable queues.
        def x2_fetch(pr, dt):
            xt = xs.tile([128, 256], BF, tag="x", name=f"x{pr}_{dt}")
            eng = (nc.sync, nc.scalar, nc.gpsimd)[dt % 3]
            eng.dma_start(
                xt[:], xT[dt * 128: (dt + 1) * 128, pr * 256: (pr + 1) * 256]
            )
            return xt

        xcur = [x2_fetch(0, dt) for dt in range(NDT)]
        xnext = []

        pending = None
        rot = None
        for rb in range(NRB):
            # rope tables for this rb's tail (used one rb later); cc and ss
            # interleaved host-side into one table -> a single DMA per rb
            ccs = cs.tile([128, 512], BF, tag="cc", name=f"cc{rb}")
            nc.sync.dma_start(ccs[:], ccR[:, rb * 512: (rb + 1) * 512])
            cct, sst = ccs[:, 0:256], ccs[:, 256:512]

            if rb % 2 == 0 and rb > 0:
                xcur = xnext
                xnext = []

            ps_q = psb.tile([128, 512], F32, tag="b", name=f"pq{rb}")
            ps_kv = psb.tile([128, 256], F32, tag="b", name=f"pkv{rb}")
            half = (rb % 2) * 128
            for dt in range(NDT):
                xt = xcur[dt][:, half: half + 128]
                if rb % 2 == 1 and rb + 1 < NRB:
                    xnext.append(x2_fetch((rb + 1) // 2, dt))
                st, sp = dt == 0, dt == NDT - 1
                nc.tensor.matmul(
                    ps_q[:], xt[:], w_sb[:, dt * 768: dt * 768 + 512],
                    start=st, stop=sp,
                )
                nc.tensor.matmul(
                    ps_kv[:], xt[:], w_sb[:, dt * 768 + 512: dt * 768 + 768],
                    start=st, stop=sp,
                )
                if dt == 2 and pending is not None:
                    prb, pq, pkv, pcc, pss = pending
                    rot = (prb, b_rope_tail_q(prb, pq, pcc, pss),
                           b_rope_tail_kv(prb, pkv, pcc, pss))
                    pending = None
                if dt == 12 and rot is not None:
                    b_transpose_tail_q(rot[0], rot[1])
                    b_transpose_tail_kv(rot[0], rot[2])
                    rot = None
                    # stripe boundary: after the transposes of the last rb
                    # of stripe g land, group g's chains become runnable
                    if rb % 4 == 0 and rb >= 4:
                        g = rb // 4 - 1
                        pending_actions.append(group_actions(g // 4, g % 4))
                if dt in INJ:
                    # drain faster when backlogged (late, larger groups)
                    pump(2 if len(pending_actions) > 1 or rb >= 24 else 1)
            pending = (rb, ps_q, ps_kv, cct, sst)

        # flush the last rb's tails
        prb, pq, pkv, pcc, pss = pending
        rot = (prb, b_rope_tail_q(prb, pq, pcc, pss),
               b_rope_tail_kv(prb, pkv, pcc, pss))
        b_transpose_tail_q(rot[0], rot[1])
        b_transpose_tail_kv(rot[0], rot[2])

        # drain any carryover, then the tail group (b1, ci=3)
        while pending_actions:
            pump(16)

        def a2a_guard(buf, tag):
            # DRAM round-trip touching one row of every 128-row an block:
            # the write-back is a tracked writer of the collective's input,
            # so the collective cannot start until all an data is visible
            # in DRAM (guards a rare input-corruption flake).
            tok = cs.tile([16, 4], BF, tag="tok", bufs=2, name=f"tok{tag}")
            blk_ap = (
                buf[:].rearrange("(blk p) c -> blk p c", p=128)[:, 127:128, 0:4]
            )
            tok_ap = tok[:].rearrange("p (one c) -> p one c", one=1)
            nc.sync.dma_start(tok_ap, blk_ap)
            nc.sync.dma_start(blk_ap, tok_ap)

        for ha, hb in ((0, 1), (2, 3)):
            gens = [attn_chain(1, ha, 3), attn_chain(1, hb, 3)]
            alive = [True, True]
            while any(alive):
                for i, gen in enumerate(gens):
                    if alive[i]:
                        try:
                            next(gen)
                        except StopIteration:
                            alive[i] = False
            if ha == 0:
                a2a_guard(a2a_in1, "a")
                nc.gpsimd.collective_compute(
                    "AllToAll",
                    mybir.AluOpType.bypass,
                    replica_groups=[list(range(NC))],
                    ins=[a2a_in1.opt()],
                    outs=[a2a_out1.opt()],
                )
        a2a_guard(a2a_in2, "b")
        nc.gpsimd.collective_compute(
            "AllToAll",
            mybir.AluOpType.bypass,
            replica_groups=[list(range(NC))],
            ins=[a2a_in2.opt()],
            outs=[a2a_out2.opt()],
        )

        psc.release()
        psb.release()

        # ---- phase D: output projection, two accumulation passes ----
        # pass 1 covers the a2a_out1 head-tiles for all col groups (hiding
        # AllToAll#2 under ~109us of PE work), saves bf16 partials, then
        # pass 2 accumulates the a2a_out2 tiles and merges. Head tiles are
        # ordered in adjacent pairs so one DMA loads two wo row-blocks.
        ht1 = [4 * i + l for i in range(8) for l in (0, 1)]
        ht2 = [4 * i + l for i in range(8) for l in (2, 3)]
        at_sb = abuf.tile([128, 32 * 512], BF, tag="at")  # col = ht*512 + row
        part_sb = wbuf.tile([128, 32 * 512], BF, tag="w")  # reuse w_sb slot

        def load_wt2(cg, ht, queue):
            # one DMA for two adjacent 128-row wo blocks -> [128, 2x512]
            wt2 = ws.tile([128, 1024], BF, tag="wo")
            queue.dma_start(
                wt2[:].rearrange("p (two c) -> p two c", two=2),
                woT[ht * 128: ht * 128 + 256, cg * 512: (cg + 1) * 512]
                .rearrange("(two p) c -> p two c", two=2),
            )
            return wt2

        def at_load(ht, src_buf, queue):
            i, htl = ht // 4, ht % 4
            srow = (i * 2 + (htl % 2)) * 128
            queue.dma_start(
                at_sb[:, ht * 512: (ht + 1) * 512],
                src_buf[srow: srow + 128, :],
            )

        with tc.tile_pool(name="psd", bufs=8, space="PSUM") as psd:
            # hoisted wo prefetch for pass-1 cg0: these DMAs depend only on
            # woT, so they run during the AllToAll-1 wait
            wt_pre = {}
            for m in range(8):
                wt_pre[(0, m)] = load_wt2(0, ht1[2 * m], (nc.sync, nc.scalar)[m % 2])
            for idx, ht in enumerate(ht1[:4]):
                at_load(ht, a2a_out1, (nc.sync, nc.scalar)[idx % 2])
            for ht in ht2:
                # gpsimd queue: waits on AllToAll#2 w/o blocking weight loads
                at_load(ht, a2a_out2, nc.gpsimd)

            def dpass(hts, src1, pidx):
                for cg in range(8):
                    po = [
                        psd.tile([128, 512], F32, tag="d", name=f"po{pidx}_{cg}_{i}")
                        for i in range(4)
                    ]
                    wt2 = None
                    for n, ht in enumerate(hts):
                        m = n // 2
                        if n % 2 == 0:
                            wt2 = wt_pre.pop((cg, m), None)
                            if wt2 is None:
                                wt2 = load_wt2(
                                    cg, ht, (nc.sync, nc.scalar)[(cg + m) % 2]
                                )
                            # prefetch the next pair's weights ahead
                            # (a just-in-time load misses by ~1us)
                            nm = m + 1
                            ncg = cg + nm // 8
                            nm = nm % 8
                            if ncg < 8 and (ncg, nm) not in wt_pre:
                                wt_pre[(ncg, nm)] = load_wt2(
                                    ncg, hts[2 * nm],
                                    (nc.sync, nc.scalar)[(ncg + nm) % 2],
                                )
                        if pidx == 1 and cg == 0 and n + 4 < 16:
                            at_load(hts[n + 4], src1, (nc.sync, nc.scalar)[n % 2])
                        half = (n % 2) * 512
                        for rt in range(4):
                            nc.tensor.matmul(
                                po[rt][:],
                                at_sb[:, ht * 512 + rt * 128: ht * 512 + (rt + 1) * 128],
                                wt2[:, half: half + 512],
                                start=(n == 0), stop=(n == 15),
                            )
                    for rt in range(4):
                        if pidx == 1:
                            nc.scalar.activation(
                                part_sb[:, (cg * 4 + rt) * 512: (cg * 4 + rt + 1) * 512],
                                po[rt][:],
                                mybir.ActivationFunctionType.Copy,
                            )
                        else:
                            tmp = osp.tile([128, 512], BF, tag="tmp")
                            nc.scalar.activation(
                                tmp[:], po[rt][:],
                                mybir.ActivationFunctionType.Copy,
                            )
                            ot = osp.tile([128, 512], F32, tag="o")
                            nc.vector.tensor_add(
                                ot[:], tmp[:],
                                part_sb[:, (cg * 4 + rt) * 512: (cg * 4 + rt + 1) * 512],
                            )
                            nc.sync.dma_start(
                                out[rt * 128: (rt + 1) * 128,
                                    cg * 512: (cg + 1) * 512],
                                ot[:],
                            )

            dpass(ht1, a2a_out1, 1)
            for m2 in range(4):
                wt_pre[(0, m2)] = load_wt2(
                    0, ht2[2 * m2], (nc.sync, nc.scalar)[m2 % 2]
                )
            dpass(ht2, a2a_out2, 2)


def _build():
    # note: walrus's LDW-elision (--enable-ldw-opt=true) rejects the
    # interleaved chain/projection instruction stream; leave it off.
    nc = bacc.Bacc("TRN2", target_bir_lowering=False, debug=False, num_devices=NC)
    xT = nc.dram_tensor("xT", [D, BS], BF, kind="ExternalInput")
    wqkvT = nc.dram_tensor("wqkvT", [D, 768], BF, kind="ExternalInput")
    woT = nc.dram_tensor("woT", [D, D], BF, kind="ExternalInput")
    ccR = nc.dram_tensor("ccR", [128, NRB * 512], BF, kind="ExternalInput")
    mask01 = nc.dram_tensor("mask01", [128, 128], BF, kind="ExternalInput")
    onec = nc.dram_tensor("onec", [128, 1], BF, kind="ExternalInput")
    iden = nc.dram_tensor("iden", [128, 128], BF, kind="ExternalInput")
    out = nc.dram_tensor("out", [R, D], F32, kind="ExternalOutput")
    with tile.TileContext(nc) as tc:
        _emit(nc, tc, (xT, wqkvT, woT, ccR, mask01, onec, iden, out))
    nc.compile()
    return nc


_NC = None


def kernel(x, wq, wk, wv, wo, freqs_cos, freqs_sin, mask, start_pos):
    global _NC
    if _NC is None:
        _NC = _build()
    nc = _NC
    bf = ml_dtypes.bfloat16

    x = np.asarray(x, dtype=np.float32)
    xT = np.ascontiguousarray(x.reshape(BS, D).T).astype(bf)

    perm = np.concatenate([np.arange(0, HD, 2), np.arange(1, HD, 2)])
    wqTp = np.asarray(wq, np.float32).T.reshape(D, H, HD)[:, :, perm]
    wkTp = np.asarray(wk, np.float32).T.reshape(D, HKV, HD)[:, :, perm]
    wvT = np.asarray(wv, np.float32).T.reshape(D, HKV, HD)
    woT = np.ascontiguousarray(np.asarray(wo, np.float32).T).astype(bf)

    fc = np.asarray(freqs_cos, np.float32)
    fs = np.asarray(freqs_sin, np.float32)
    # row-major RoPE tables per row block, replicated x4 along free axis
    pos = (np.arange(BS) % S).reshape(NRB, 128)
    ccR4 = np.tile(fc[pos], (1, 1, 4)).transpose(1, 0, 2).reshape(128, NRB, 256)
    ssR4 = np.tile(fs[pos], (1, 1, 4)).transpose(1, 0, 2).reshape(128, NRB, 256)
    # interleave per-rb: [cc 256 | ss 256] so one DMA loads both tables
    ccR = np.ascontiguousarray(
        np.concatenate([ccR4, ssR4], axis=2).reshape(128, NRB * 512)
    ).astype(bf)

    # 0/1 causal mask for the diagonal block (krow <= qcol keeps)
    mask01 = np.where(
        np.arange(128)[:, None] > np.arange(128)[None, :], 0.0, 1.0
    ).astype(bf)
    onec = np.ones((128, 1), dtype=bf)
    iden = np.eye(128, dtype=bf)

    in_maps = []
    for c in range(NC):
        wqkv = np.concatenate(
            [
                wqTp[:, 4 * c: 4 * c + 4].reshape(D, 512),
                wkTp[:, c],
                wvT[:, c],
            ],
            axis=1,
        ).astype(bf)
        in_maps.append(
            {
                "xT": xT,
                "wqkvT": np.ascontiguousarray(wqkv),
                "woT": woT,
                "ccR": ccR,
                "mask01": mask01,
                "onec": onec,
                "iden": iden,
            }
        )

    res = bass_utils.run_bass_kernel_spmd(
        nc, in_maps, core_ids=list(range(NC)), trace=PROFILE, tmpdir=TMPDIR
    )
    if PROFILE:
        print(f"HW exec time: {res.exec_time_ns} ns")
        if res.instructions_and_trace is not None:
            print(f"trace: {res.instructions_and_trace[1]}")

    out_full = np.empty((BS, D), dtype=np.float32)
    for c in range(NC):
        out_full[R * c: R * (c + 1)] = res.results[c]["out"]
    return out_full.reshape(B, S, D)
